# revision 41
# baseline (speedup 1.0000x reference)
"""Trainium2 Bass kernel for quadratic non-softmax attention:

    X[n,c,i] = sum_j exp(a[n,c,i] * b[n,c,j]) * v[n,c,j]

with a=fxA, b=fyA, v=fyB, each (2,16,32,32) fp32 -> 32 independent
(n,c) slices of an HWxHW=1024x1024 problem. Sharded 4 slices/core
across 8 NeuronCores (pure data parallel).

Two implementations:

v1 (direct): per i-tile, ScalarE computes E[p,j]=Exp(a[i_p]*b[j]) with a
per-partition `scale`, VectorE's fused affine_mul_reduce does
sum_j E[p,j]*v[j]. ~32 ScalarE exp passes of 1024 elements per core.

v2 (grid-factored, default): quantize a_i = g_k + r_i on a 64-point grid
g_k=(k-32)*h, h=5/32 (exact in fp32, so r is computed exactly). Then
  exp(a_i b_j) = exp(g_k b_j) * exp(r_i b_j)
  X[i] = sum_d r_i^d * H_d[k_i],  H_d[k] = sum_j e^{g_k b_j} v_j b_j^d/d!
(degree-7 Taylor in r*b, |r*b| <= (h/2)*max|b| ~ 0.37 -> trunc err ~8e-9
relative). ScalarE only computes the 128x64 table exp(b_j*g_k) per
j-tile (16x less exp work); H_d comes from one accumulated fp32 matmul
per j-tile; the per-i gather H[k_i,:] is a one-hot fp32 matmul; the
Taylor evaluation is an elementwise multiply with precomputed r^d powers
plus a free-axis reduce_sum on VectorE.
"""

import os

import numpy as np

import concourse.bass as bass
import concourse.mybir as mybir
import concourse.tile as tile
from concourse import bacc
from concourse.bass_utils import run_bass_kernel_spmd

N_CORES = 8
N_SLICES = 32          # N*C = 2*16
S = N_SLICES // N_CORES  # 4 slices per core
HW = 1024              # H*W = 32*32
P = 128                # partitions
NT = HW // P           # 8 tiles of 128
OUT_SHAPE = (2, 16, 32, 32)
F32 = mybir.dt.float32
I32 = mybir.dt.int32

GRID_K = 64            # grid points (= H matmul output partitions)
GRID_H = 5.0 / 32.0    # grid step; exact in fp32 so r = a - k*h is exact
GRID_LO = -5.0         # grid value of index 0; covers |a| <= 5 + h/2
D = 8                  # Taylor terms d=0..7; |r*b| <= (h/2)*4.7 ~ 0.37,
                       # truncation |rb|^8/8! ~ 8e-9 << fp32 noise
BF16 = mybir.dt.bfloat16


def _new_nc():
    return bacc.Bacc(
        "TRN2",
        target_bir_lowering=False,
        debug=False,
        num_devices=N_CORES,
    )


def build_nc_v1() -> bass.Bass:
    nc = _new_nc()
    a = nc.dram_tensor("a", [S, HW], F32, kind="ExternalInput")
    b = nc.dram_tensor("b", [S, HW], F32, kind="ExternalInput")
    v = nc.dram_tensor("v", [S, HW], F32, kind="ExternalInput")
    x = nc.dram_tensor("x", [S, HW], F32, kind="ExternalOutput")

    with tile.TileContext(nc) as tc:
        with (
            tc.tile_pool(name="bcast", bufs=2) as bcast,
            tc.tile_pool(name="small", bufs=2) as small,
            tc.tile_pool(name="work", bufs=4) as work,
        ):
            for s in range(S):
                acol = small.tile([P, NT], F32, tag="acol")
                nc.sync.dma_start(
                    out=acol, in_=a[s].rearrange("(p t) -> p t", p=P)
                )
                bb = bcast.tile([P, HW], F32, tag="bb")
                nc.sync.dma_start(
                    out=bb, in_=b[s : s + 1, :].to_broadcast((P, HW))
                )
                vb = bcast.tile([P, HW], F32, tag="vb")
                nc.sync.dma_start(
                    out=vb, in_=v[s : s + 1, :].to_broadcast((P, HW))
                )
                xcol = small.tile([P, NT], F32, tag="xcol")
                for t in range(NT):
                    e = work.tile([P, HW], F32, tag="e")
                    nc.scalar.activation(
                        out=e,
                        in_=bb,
                        func=mybir.ActivationFunctionType.Exp,
                        scale=acol[:, t : t + 1],
                    )
                    # prod = e * vb ; xcol[:, t] = sum_j prod[:, j]
                    prod = work.tile([P, HW], F32, tag="prod")
                    nc.vector.affine_mul_reduce(
                        out=prod,
                        accum_out=xcol[:, t : t + 1],
                        in0=e,
                        in1=vb,
                        scale=1.0,
                        bias=0.0,
                    )
                nc.sync.dma_start(
                    out=x[s].rearrange("(p t) -> p t", p=P), in_=xcol
                )
    nc.compile()
    return nc


def build_nc_v2() -> bass.Bass:
    nc = _new_nc()
    a = nc.dram_tensor("a", [S, HW], F32, kind="ExternalInput")
    b = nc.dram_tensor("b", [S, HW], F32, kind="ExternalInput")
    v = nc.dram_tensor("v", [S, HW], F32, kind="ExternalInput")
    x = nc.dram_tensor("x", [S, HW], F32, kind="ExternalOutput")

    # index mapping: i = p*NT + t (and j = p*NT + u) so the DRAM side of
    # every strided DMA moves NT contiguous elements per (partition, slice)
    col_view = lambda t: t.rearrange("s (p t) -> p s t", p=P)

    with tile.TileContext(nc) as tc:
        with (
            tc.tile_pool(name="const", bufs=1) as const,
            tc.tile_pool(name="cols", bufs=1) as cols,
            tc.tile_pool(name="gt", bufs=8) as gtp,
            tc.tile_pool(name="oh", bufs=1) as ohp,
            tc.tile_pool(name="hsb", bufs=2) as hsbp,
            tc.tile_pool(name="hps", bufs=2, space="PSUM") as hps,
            tc.tile_pool(name="coefps", bufs=1, space="PSUM") as coefps,
            tc.tile_pool(name="dram", bufs=1, space="DRAM") as dram,
        ):
            # ---- constants ----
            io32 = const.tile([P, GRID_K], I32, tag="io32")
            nc.gpsimd.iota(
                out=io32, pattern=[[1, GRID_K]], base=0, channel_multiplier=0
            )
            iof = const.tile([P, GRID_K], F32, tag="iof")
            nc.vector.tensor_copy(out=iof, in_=io32)
            # gridb[p, k] = k*h + GRID_LO, same on every partition (exact fp32)
            gridb = const.tile([P, GRID_K], F32, tag="gridb")
            nc.scalar.activation(
                out=gridb,
                in_=iof,
                func=mybir.ActivationFunctionType.Copy,
                scale=GRID_H,
                bias=GRID_LO,
            )
            pc32 = const.tile([GRID_K, 1], I32, tag="pc32")
            nc.gpsimd.iota(out=pc32, pattern=[[0, 1]], base=0, channel_multiplier=1)
            pcf = const.tile([GRID_K, 1], F32, tag="pcf")
            nc.vector.tensor_copy(out=pcf, in_=pc32)

            # ---- batched input loads (column layout: [p, s, t]) ----
            acol = cols.tile([P, S, NT], F32, tag="acol")
            nc.sync.dma_start(out=acol, in_=col_view(a))
            bcol = cols.tile([P, S, NT], F32, tag="bcol")
            nc.sync.dma_start(out=bcol, in_=col_view(b))
            vcol = cols.tile([P, S, NT], F32, tag="vcol")
            nc.sync.dma_start(out=vcol, in_=col_view(v))

            # ---- grid index + remainder (column layout, all slices) ----
            # a*(1/h) with int32 output dtype = round-to-nearest in one op
            ki32 = cols.tile([P, S, NT], I32, tag="ki32")
            nc.vector.tensor_scalar_mul(out=ki32, in0=acol, scalar1=1.0 / GRID_H)
            kf = cols.tile([P, S, NT], F32, tag="kf")
            nc.vector.tensor_copy(out=kf, in_=ki32)
            # r = a - kf*h  (exact: kf*h is exact for h=5/32, |kf|<=32)
            rc = cols.tile([P, S, NT], F32, tag="rc")
            nc.vector.scalar_tensor_tensor(
                out=rc, in0=kf, scalar=-GRID_H, in1=acol,
                op0=mybir.AluOpType.mult, op1=mybir.AluOpType.add,
            )

            # grid index (0..GRID_K-1) = ki + 32; small ints are exact in bf16
            kis = cols.tile([P, S, NT], BF16, tag="kis")
            nc.vector.tensor_scalar_add(out=kis, in0=ki32, scalar1=-GRID_LO / GRID_H)
            # roundtrip through DRAM to get row layout for the one-hot compare
            kinl = dram.tile([S, HW], BF16, tag="kinl")
            nc.sync.dma_start(out=col_view(kinl), in_=kis)

            # ---- Taylor weights W[p, s, t, d] = v * b^d / d! ----
            W = cols.tile([P, S, NT, D], F32, tag="W")
            nc.vector.tensor_copy(out=W[:, :, :, 0], in_=vcol)
            for d in range(1, D):
                # W_d = (W_{d-1} * (1/d)) * b
                nc.vector.scalar_tensor_tensor(
                    out=W[:, :, :, d],
                    in0=W[:, :, :, d - 1],
                    scalar=1.0 / d,
                    in1=bcol,
                    op0=mybir.AluOpType.mult,
                    op1=mybir.AluOpType.mult,
                )

            # ---- powers RD[p, s, t, d] = r^d (early; off the critical path) ----
            RD = cols.tile([P, S, NT, D], F32, tag="RD")
            nc.vector.memset(RD[:, :, :, 0], 1.0)
            for d in range(1, D):
                nc.vector.tensor_mul(
                    out=RD[:, :, :, d], in0=RD[:, :, :, d - 1], in1=rc
                )

            coef = coefps.tile([P, S, NT, D], F32, tag="coef")

            # ---- one-hot rows for every slice, computed up front so the
            # gather matmuls never wait on the kib broadcast DMAs ----
            ohs = []
            for s in range(S):
                kib = ohp.tile([GRID_K, HW], BF16, tag=f"kib{s}")
                nc.sync.dma_start(
                    out=kib, in_=kinl[s : s + 1, :].to_broadcast((GRID_K, HW))
                )
                oh = ohp.tile([GRID_K, HW], F32, tag=f"oh{s}")
                nc.vector.tensor_scalar(
                    out=oh,
                    in0=kib,
                    scalar1=pcf,
                    scalar2=None,
                    op0=mybir.AluOpType.is_equal,
                )
                ohs.append(oh)

            for s in range(S):
                # ---- H_d[k] = sum_j e^{g_k b_j} v_j b_j^d/d!  (PSUM accum) ----
                Hps = hps.tile([GRID_K, D], F32, tag="H")
                for u in range(NT):
                    gt = gtp.tile([P, GRID_K], F32, tag="gt")
                    nc.scalar.activation(
                        out=gt,
                        in_=gridb,
                        func=mybir.ActivationFunctionType.Exp,
                        scale=bcol[:, s, u : u + 1],
                    )
                    nc.tensor.matmul(
                        out=Hps,
                        lhsT=gt,
                        rhs=W[:, s, u, :],
                        start=(u == 0),
                        stop=(u == NT - 1),
                    )
                Hsb = hsbp.tile([GRID_K, D], F32, tag="Hsb")
                nc.vector.tensor_copy(out=Hsb, in_=Hps)

                # ---- one-hot gather: coef[i, :] = H[k_i, :] ----
                # free position in oh is i = c*NT + u; tile u gathers the
                # i's congruent to u (strided columns), matching coef[c,s,u]
                oh_r = ohs[s].rearrange("p (c t) -> p t c", t=NT)
                for u in range(NT):
                    nc.tensor.matmul(
                        out=coef[:, s, u, :],
                        lhsT=oh_r[:, u, :],
                        rhs=Hsb,
                        start=True,
                        stop=True,
                    )

            # ---- X = sum_d r^d * H_d[k_i]: one multiply + one reduce ----
            prodc = cols.tile([P, S, NT, D], F32, tag="prodc")
            nc.vector.tensor_mul(out=prodc, in0=RD, in1=coef)
            xall = cols.tile([P, S, NT], F32, tag="xall")
            nc.vector.reduce_sum(out=xall, in_=prodc, axis=mybir.AxisListType.X)
            nc.sync.dma_start(out=col_view(x), in_=xall)
    nc.compile()
    return nc


_NC_CACHE = {}
_VERSION = os.environ.get("KERNEL_VERSION", "v2")


def _get_nc():
    ver = _VERSION
    if ver not in _NC_CACHE:
        _NC_CACHE[ver] = build_nc_v2() if ver == "v2" else build_nc_v1()
    return _NC_CACHE[ver]


def kernel(fxA, fyA, fyB, _trace=False, _tmpdir=None):
    a_full = np.ascontiguousarray(np.asarray(fxA), dtype=np.float32).reshape(
        N_SLICES, HW
    )
    b_full = np.ascontiguousarray(np.asarray(fyA), dtype=np.float32).reshape(
        N_SLICES, HW
    )
    v_full = np.ascontiguousarray(np.asarray(fyB), dtype=np.float32).reshape(
        N_SLICES, HW
    )

    in_maps = []
    for c in range(N_CORES):
        lo, hi = c * S, (c + 1) * S
        in_maps.append(
            {"a": a_full[lo:hi], "b": b_full[lo:hi], "v": v_full[lo:hi]}
        )

    res = run_bass_kernel_spmd(
        _get_nc(),
        in_maps,
        core_ids=list(range(N_CORES)),
        trace=_trace,
        tmpdir=_tmpdir,
    )
    out = np.concatenate([r["x"] for r in res.results], axis=0)
    if _trace:
        kernel.last_results = res
    return out.reshape(OUT_SHAPE).astype(np.float32)
